# revision 1
# baseline (speedup 1.0000x reference)
"""Trainium2 Bass kernel for tf-idf weighted embedding pooling + MLP.

Math: per batch row b (64 rows), tf[b,s] = within-row count of token x[s,b];
scores = where(tok==0, 0, tf*idf[tok]); pooled[b] = sum_s scores * emb[tok];
out = softmax(relu(relu(pooled@W1.T+b1)@W2.T+b2)@W3.T+b3).

Key identity: pooled[b] = sum_t c_t^2 * idf_t * emb_t  (c_t = count of t in row b).
Histogram per row is computed on the PE as a one-hot digit matmul:
  tok = hi*256 + lo,  H[lo, hi] = OneHotLo^T @ OneHotHi  (accumulated over s)
so H[lo, hi] = count of token hi*256+lo.  Then a[lo, hi] = H^2 * idfT[lo, hi].

Phase 2 (default): vocab-sharded pooled matmul. Each core histograms its own
8 batch rows, AllToAll redistributes the a-vectors so core c holds a[all 64
rows, vocab shard c], each core contracts its 6400-row emb shard (zero-padded
past 50000) against a for all 64 rows, and a ReduceScatter hands core c the
final pooled rows 8c..8c+8 for its MLP slice. Per-core HBM read of emb drops
from 51.2MB to 6.4MB.

Phase 1 (fallback): every core streams the full 51.2MB emb table.
"""

import os
import sys

import numpy as np

sys.path.insert(0, "/opt/trn_rl_repo")

import concourse.bass as bass  # noqa: E402,F401
import concourse.mybir as mybir  # noqa: E402
import concourse.tile as tile  # noqa: E402
from concourse import bacc  # noqa: E402
from concourse.masks import make_identity  # noqa: E402

P = 128
S = 2048
B = 64
D = 256
V = 50000
NCORES = 8
RPC = B // NCORES  # rows per core
NHI = 196  # ceil(50000/256)
NLO = 256
STILES = S // P  # 16
VPAD = NHI * NLO  # 50176
NHL = 25  # hi rows per vocab shard (200 padded hi rows / 8)
VSH = NHL * NLO  # 6400 vocab rows per shard

F32 = mybir.dt.float32
BF16 = mybir.dt.bfloat16
I32 = mybir.dt.int32

_CACHE = {}


def _mlp_tail(nc, tc, cpool, ps_mlp, pooled_sb, identity,
              w1t_sb, b1_sb, w2t_sb, b2a_sb, b2b_sb, w3a_sb, w3b_sb, b3_sb, out):
    """pooled_sb [RPC, 256] -> softmax out DMA."""
    pooledT = cpool.tile([P, 2, RPC], F32, tag="pooledT")
    for kc in range(2):
        ptp = ps_mlp.tile([P, RPC], F32, tag="ptp")
        nc.tensor.transpose(
            ptp[:, :], pooled_sb[:, kc * P : (kc + 1) * P], identity[:RPC, :RPC]
        )
        nc.vector.tensor_copy(pooledT[:, kc, :], ptp[:, :])

    h1_ps = ps_mlp.tile([100, RPC], F32, tag="h1")
    for kc in range(2):
        nc.tensor.matmul(
            h1_ps[:, :], lhsT=w1t_sb[:, kc, :], rhs=pooledT[:, kc, :],
            start=(kc == 0), stop=(kc == 1),
        )
    h1_sb = cpool.tile([100, RPC], F32, tag="h1_sb")
    nc.scalar.activation(
        h1_sb[:], h1_ps[:, :], mybir.ActivationFunctionType.Relu,
        bias=b1_sb[:, 0:1], scale=1.0,
    )

    h2a_ps = ps_mlp.tile([P, RPC], F32, tag="h2a")
    nc.tensor.matmul(h2a_ps[:, :], lhsT=w2t_sb[:, 0:128], rhs=h1_sb[:, :],
                     start=True, stop=True)
    h2b_ps = ps_mlp.tile([22, RPC], F32, tag="h2b")
    nc.tensor.matmul(h2b_ps[:, :], lhsT=w2t_sb[:, 128:150], rhs=h1_sb[:, :],
                     start=True, stop=True)
    h2a_sb = cpool.tile([P, RPC], F32, tag="h2a_sb")
    h2b_sb = cpool.tile([22, RPC], F32, tag="h2b_sb")
    nc.scalar.activation(h2a_sb[:], h2a_ps[:, :],
                         mybir.ActivationFunctionType.Relu,
                         bias=b2a_sb[:, 0:1], scale=1.0)
    nc.scalar.activation(h2b_sb[:], h2b_ps[:, :],
                         mybir.ActivationFunctionType.Relu,
                         bias=b2b_sb[:, 0:1], scale=1.0)

    lg_ps = ps_mlp.tile([2, RPC], F32, tag="lg")
    nc.tensor.matmul(lg_ps[:, :], lhsT=w3a_sb[:, :], rhs=h2a_sb[:, :],
                     start=True, stop=False)
    nc.tensor.matmul(lg_ps[:, :], lhsT=w3b_sb[:, :], rhs=h2b_sb[:, :],
                     start=False, stop=True)
    lg_sb = cpool.tile([2, RPC], F32, tag="lg_sb")
    nc.scalar.add(lg_sb[:], lg_ps[:, :], b3_sb[:, 0:1])

    lt_ps = ps_mlp.tile([RPC, 2], F32, tag="lt")
    nc.tensor.transpose(lt_ps[:, :], lg_sb[:, :], identity[:2, :2])
    e_sb = cpool.tile([RPC, 2], F32, tag="e_sb")
    nc.scalar.activation(e_sb[:], lt_ps[:, :], mybir.ActivationFunctionType.Exp)
    ssum = cpool.tile([RPC, 1], F32, tag="ssum")
    nc.vector.tensor_reduce(ssum[:], e_sb[:], axis=mybir.AxisListType.X,
                            op=mybir.AluOpType.add)
    rinv = cpool.tile([RPC, 1], F32, tag="rinv")
    nc.vector.reciprocal(rinv[:], ssum[:])
    res_sb = cpool.tile([RPC, 2], F32, tag="res_sb")
    nc.vector.tensor_scalar(out=res_sb[:], in0=e_sb[:], scalar1=rinv[:, 0:1],
                            scalar2=None, op0=mybir.AluOpType.mult)
    nc.sync.dma_start(out[:, :], res_sb[:])


def _build_nc(phase=1, reps=1):
    nc = bacc.Bacc(None, target_bir_lowering=False, debug=False)

    xt = nc.dram_tensor("xt", [RPC, S], I32, kind="ExternalInput")
    if phase == 1:
        emb = nc.dram_tensor("emb", [V, D], F32, kind="ExternalInput")
    else:
        embs = nc.dram_tensor("embs", [VSH, D], F32, kind="ExternalInput")
    idf_t = nc.dram_tensor("idf_t", [NLO, NHI], F32, kind="ExternalInput")
    w1t = nc.dram_tensor("w1t", [256, 100], F32, kind="ExternalInput")
    b1 = nc.dram_tensor("b1", [100], F32, kind="ExternalInput")
    w2t = nc.dram_tensor("w2t", [100, 150], F32, kind="ExternalInput")
    b2 = nc.dram_tensor("b2", [150], F32, kind="ExternalInput")
    w3t = nc.dram_tensor("w3t", [150, 2], F32, kind="ExternalInput")
    b3 = nc.dram_tensor("b3", [2], F32, kind="ExternalInput")
    out = nc.dram_tensor("out", [RPC, 2], F32, kind="ExternalOutput")

    with tile.TileContext(nc) as tc:
        with (
            tc.tile_pool(name="const", bufs=1) as cpool,
            tc.tile_pool(name="work", bufs=3) as wpool,
            tc.tile_pool(name="oh", bufs=6) as ohpool,
            tc.tile_pool(name="embp", bufs=17) as embpool,
            tc.tile_pool(name="arp", bufs=6) as arpool,
            tc.tile_pool(name="dram", bufs=2, space="DRAM") as dpool,
            tc.tile_pool(name="ps_acc", bufs=2, space="PSUM") as ps_acc,
        ):
            # ---------- constants ----------
            iota_i32 = cpool.tile([P, NLO], I32)
            nc.gpsimd.iota(iota_i32[:], pattern=[[1, NLO]], base=0,
                           channel_multiplier=0)
            iota_bf = cpool.tile([P, NLO], BF16)
            nc.vector.tensor_copy(iota_bf[:], iota_i32[:])

            identity = cpool.tile([P, P], F32)
            make_identity(nc, identity[:])

            idf_sb = cpool.tile([P, 2, NHI], F32)
            nc.sync.dma_start(idf_sb[:, 0, :], idf_t[0:128, :])
            nc.sync.dma_start(idf_sb[:, 1, :], idf_t[128:256, :])

            w1t_sb = cpool.tile([P, 2, 100], F32)
            nc.sync.dma_start(w1t_sb[:, :, :],
                              w1t[:, :].rearrange("(c p) m -> p c m", p=P))
            b1_sb = cpool.tile([100, 1], F32)
            nc.sync.dma_start(b1_sb[:, :], b1[:, None])
            w2t_sb = cpool.tile([100, 150], F32)
            nc.sync.dma_start(w2t_sb[:, :], w2t[:, :])
            b2a_sb = cpool.tile([128, 1], F32)
            b2b_sb = cpool.tile([22, 1], F32)
            nc.sync.dma_start(b2a_sb[:, :], b2[:128, None])
            nc.sync.dma_start(b2b_sb[:, :], b2[128:150, None])
            w3a_sb = cpool.tile([128, 2], F32)
            w3b_sb = cpool.tile([22, 2], F32)
            nc.sync.dma_start(w3a_sb[:, :], w3t[0:128, :])
            nc.sync.dma_start(w3b_sb[:, :], w3t[128:150, :])
            b3_sb = cpool.tile([2, 1], F32)
            nc.sync.dma_start(b3_sb[:, :], b3[:, None])

            for _rep in range(reps):
                # ---------- tokens: [128, RPC*16], s = p*16 + f per row ------
                tok_i32 = cpool.tile([P, RPC * STILES], I32, tag="tok", bufs=2)
                for r in range(RPC):
                    nc.sync.dma_start(
                        tok_i32[:, r * STILES : (r + 1) * STILES],
                        xt[r, :].rearrange("(p f) -> p f", p=P),
                    )
                lo_i32 = wpool.tile([P, RPC * STILES], I32, tag="lo_i32")
                hi_i32 = wpool.tile([P, RPC * STILES], I32, tag="hi_i32")
                nc.vector.tensor_scalar(
                    out=lo_i32[:], in0=tok_i32[:], scalar1=255, scalar2=None,
                    op0=mybir.AluOpType.bitwise_and)
                nc.vector.tensor_scalar(
                    out=hi_i32[:], in0=tok_i32[:], scalar1=8, scalar2=None,
                    op0=mybir.AluOpType.logical_shift_right)
                lo_f = cpool.tile([P, RPC * STILES], F32, tag="lo_f", bufs=2)
                hi_f = cpool.tile([P, RPC * STILES], F32, tag="hi_f", bufs=2)
                nc.vector.tensor_copy(lo_f[:], lo_i32[:])
                nc.vector.tensor_copy(hi_f[:], hi_i32[:])
                # negated hi for the ACT-engine one-hot (bias port)
                hi_neg = cpool.tile([P, RPC * STILES], F32, tag="hi_neg",
                                    bufs=2)
                nc.vector.tensor_scalar(
                    out=hi_neg[:], in0=hi_f[:], scalar1=-1.0, scalar2=None,
                    op0=mybir.AluOpType.mult)

                # ---------- per-row histograms -> a = H^2 * idfT ----------
                a_all = [cpool.tile([P, NHI, RPC], F32, name=f"a_all{mh}",
                                    tag=f"a_all{mh}", bufs=2)
                         for mh in range(2)]
                if phase == 2:
                    # AllToAll buffers: [hi(200=8sh*25), mh, p, r]
                    a2a_in = dpool.tile([8 * NHL, 2, P, RPC], F32,
                                        tag="a2a_in")
                    a2a_out = dpool.tile([8, NHL, 2, P, RPC], F32,
                                         tag="a2a_out")
                    # zero the hi 196..199 pad region (uninit DRAM garbage
                    # would ride the AllToAll and NaN-poison the matmul)
                    zpad = cpool.tile([P, (8 * NHL - NHI) * 2 * RPC], F32,
                                      tag="zpad", bufs=2)
                    nc.vector.memset(zpad[:], 0.0)
                    nc.sync.dma_start(
                        a2a_in[NHI:, :, :, :].rearrange(
                            "hi mh p r -> p hi mh r"),
                        zpad[:].rearrange("p (hi mh r) -> p hi mh r",
                                          hi=8 * NHL - NHI, mh=2),
                    )

                with tc.tile_pool(name="ps_ht", bufs=2, space="PSUM") as ps_ht:
                    for r in range(RPC):
                        ht_ps = [ps_ht.tile([P, NHI], F32, name=f"ht{mh}",
                                            tag=f"ht{mh}")
                                 for mh in range(2)]
                        for f in range(STILES):
                            col = r * STILES + f
                            lo_oh = ohpool.tile([P, NLO], BF16, tag="lo_oh")
                            hi_oh = ohpool.tile([P, NHI], BF16, tag="hi_oh")
                            if True:
                                nc.vector.tensor_scalar(
                                    out=lo_oh[:], in0=iota_bf[:],
                                    scalar1=lo_f[:, col : col + 1],
                                    scalar2=None,
                                    op0=mybir.AluOpType.is_equal)
                                d2 = ohpool.tile([P, NHI], F32,
                                                 tag="d2")
                                nc.scalar.activation(
                                    d2[:], iota_bf[:, :NHI],
                                    mybir.ActivationFunctionType.Square,
                                    bias=hi_neg[:, col : col + 1], scale=1.0)
                                nc.scalar.activation(
                                    hi_oh[:], d2[:],
                                    mybir.ActivationFunctionType.Relu,
                                    bias=1.0, scale=-1.0)
                            for mh in range(2):
                                nc.tensor.matmul(
                                    ht_ps[mh][:, :],
                                    lhsT=lo_oh[:, mh * P : (mh + 1) * P],
                                    rhs=hi_oh[:, :],
                                    start=(f == 0), stop=(f == STILES - 1))
                        for mh in range(2):
                            sq = wpool.tile([P, NHI], F32, tag="sq")
                            nc.scalar.square(sq[:], ht_ps[mh][:, :])
                            nc.vector.tensor_tensor(
                                out=a_all[mh][:, :, r], in0=sq[:],
                                in1=idf_sb[:, mh, :],
                                op=mybir.AluOpType.mult)

                if phase == 2:
                    # bulk-write a to the exchange buffer: one DMA per mh,
                    # innermost r contiguous (32B runs)
                    for mh in range(2):
                        nc.sync.dma_start(
                            a2a_in[:NHI, mh, :, :].rearrange(
                                "hi p r -> p hi r"),
                            a_all[mh][:, :, :],
                        )

                if phase == 1:
                    # ------- pooled over full-vocab chunks (M=RPC) -------
                    pooled_ps = ps_acc.tile([RPC, D], F32, tag="pooled")
                    chunks = []
                    v = 0
                    while v + 1024 <= 49152:
                        chunks.append((v, 1024))
                        v += 1024
                    chunks.append((49152, 768))
                    chunks.append((49920, 80))
                    n_mm = sum((n + 127) // 128 for _, n in chunks)
                    mm_i = 0
                    for v0, n in chunks:
                        embc = embpool.tile([P, 8, D], F32, tag="embc")
                        if n >= P:
                            nsub = n // P
                            nc.sync.dma_start(
                                embc[:, :nsub, :],
                                emb[v0 : v0 + nsub * P, :].rearrange(
                                    "(c p) d -> p c d", p=P))
                        else:
                            nsub = 1
                            nc.sync.dma_start(embc[:n, 0, :],
                                              emb[v0 : v0 + n, :])
                        for c in range(nsub):
                            vv = v0 + c * P
                            kk = min(P, n - c * P)
                            hi = vv >> 8
                            mh = (vv >> 7) & 1
                            nc.tensor.matmul(
                                pooled_ps[:, :],
                                lhsT=a_all[mh][:kk, hi, :],
                                rhs=embc[:kk, c, :],
                                start=(mm_i == 0), stop=(mm_i == n_mm - 1))
                            mm_i += 1
                    pooled_sb = cpool.tile([RPC, D], F32, tag="pooled_sb")
                    nc.vector.tensor_copy(pooled_sb[:], pooled_ps[:, :])
                else:
                    # ------- AllToAll, shard matmul (M=64), ReduceScatter ---
                    nc.gpsimd.collective_compute(
                        "AllToAll", mybir.AluOpType.bypass,
                        replica_groups=[list(range(NCORES))],
                        ins=[a2a_in[:, :, :, :]],
                        outs=[a2a_out[:, :, :, :, :]],
                    )
                    pooled_ps = ps_acc.tile([B, D], F32, tag="pooled")
                    # emb shard chunks of 1024 rows (6 full + 1x256)
                    ech = [(i * 1024, 1024) for i in range(6)] + [(6144, 256)]
                    mm_i = 0
                    for v0, n in ech:
                        embc = embpool.tile([P, 8, D], F32, tag="embc")
                        nsub = n // P
                        nc.sync.dma_start(
                            embc[:, :nsub, :],
                            embs[v0 : v0 + n, :].rearrange(
                                "(c p) d -> p c d", p=P))
                        for c in range(nsub):
                            k = v0 // P + c  # 0..49
                            hl, mh = k >> 1, k & 1
                            ar = arpool.tile([P, B], F32, tag="ar")
                            nc.sync.dma_start(
                                ar[:].rearrange("p (j r) -> p j r", j=8),
                                a2a_out[:, hl, mh, :, :].rearrange(
                                    "j p r -> p j r"))
                            nc.tensor.matmul(
                                pooled_ps[:, :], lhsT=ar[:],
                                rhs=embc[:, c, :],
                                start=(mm_i == 0), stop=(mm_i == 49))
                            mm_i += 1
                    pooled_full = cpool.tile([B, D], F32, tag="pooled_full",
                                             bufs=2)
                    nc.vector.tensor_copy(pooled_full[:], pooled_ps[:, :])
                    rs_in = dpool.tile([B, D], F32, tag="rs_in")
                    rs_out = dpool.tile([RPC, D], F32, tag="rs_out")
                    nc.sync.dma_start(rs_in[:, :], pooled_full[:])
                    nc.gpsimd.collective_compute(
                        "ReduceScatter", mybir.AluOpType.add,
                        replica_groups=[list(range(NCORES))],
                        ins=[rs_in[:, :]],
                        outs=[rs_out[:, :]],
                    )
                    pooled_sb = cpool.tile([RPC, D], F32, tag="pooled_sb")
                    nc.sync.dma_start(pooled_sb[:], rs_out[:, :])

                # ---------- MLP + softmax on own 8 rows ----------
                with tc.tile_pool(name="ps_mlp", bufs=1,
                                  space="PSUM") as ps_mlp:
                    _mlp_tail(nc, tc, cpool, ps_mlp, pooled_sb, identity,
                              w1t_sb, b1_sb, w2t_sb, b2a_sb, b2b_sb,
                              w3a_sb, w3b_sb, b3_sb, out)

    nc.compile()
    return nc


def _get_nc(phase=1, reps=1):
    key = f"nc_p{phase}_r{reps}"
    if key not in _CACHE:
        _CACHE[key] = _build_nc(phase, reps)
    return _CACHE[key]


class _Runner:
    """Cached jitted shard_map over the NEFF custom call (mirrors
    bass2jax.run_bass_via_pjrt, but reusable with device-resident inputs)."""

    def __init__(self, nc):
        import jax
        from jax.experimental.shard_map import shard_map
        from jax.sharding import Mesh, NamedSharding, PartitionSpec

        from concourse import bass2jax

        bass2jax.install_neuronx_cc_hook()
        assert nc.dbg_addr is None
        partition_name = (
            nc.partition_id_tensor.name if nc.partition_id_tensor else None
        )
        self._nc = nc
        self._partition_name = partition_name

        self.jax = jax
        in_names, out_names, out_avals, zero_outs = [], [], [], []
        for alloc in nc.m.functions[0].allocations:
            if not isinstance(alloc, mybir.MemoryLocationSet):
                continue
            name = alloc.memorylocations[0].name
            if alloc.kind == "ExternalInput":
                if name == partition_name:
                    continue
                in_names.append(name)
            elif alloc.kind == "ExternalOutput":
                out_names.append(name)
                shape = tuple(alloc.tensor_shape)
                dtype = mybir.dt.np(alloc.dtype)
                out_avals.append(jax.core.ShapedArray(shape, dtype))
                zero_outs.append(np.zeros((NCORES * shape[0], *shape[1:]), dtype))
        self.in_names = list(in_names)
        self.out_names = out_names
        self.out_avals = out_avals
        self.zero_outs = zero_outs
        n_params = len(in_names)
        n_outs = len(out_names)
        bind_names = tuple(
            in_names + out_names + ([partition_name] if partition_name else [])
        )
        donate = tuple(range(n_params, n_params + n_outs))

        def _body(*args):
            operands = list(args)
            if partition_name is not None:
                operands.append(bass2jax.partition_id_tensor())
            outs = bass2jax._bass_exec_p.bind(
                *operands,
                out_avals=tuple(out_avals),
                in_names=bind_names,
                out_names=tuple(out_names),
                lowering_input_output_aliases=(),
                sim_require_finite=True,
                sim_require_nnan=True,
                nc=nc,
            )
            return tuple(outs)

        devices = jax.devices()[:NCORES]
        self.mesh = Mesh(np.asarray(devices), ("core",))
        self.sharding = NamedSharding(self.mesh, PartitionSpec("core"))
        in_specs = (PartitionSpec("core"),) * (n_params + n_outs)
        out_specs = (PartitionSpec("core"),) * n_outs
        self.fn = jax.jit(
            shard_map(
                _body,
                mesh=self.mesh,
                in_specs=in_specs,
                out_specs=out_specs,
                check_rep=False,
            ),
            donate_argnums=donate,
            keep_unused=True,
        )

    def put_inputs(self, in_maps):
        concat = [
            np.concatenate([np.asarray(m[name]) for m in in_maps], axis=0)
            for name in self.in_names
        ]
        return [self.jax.device_put(a, self.sharding) for a in concat]

    def run(self, dev_in):
        zo = [self.jax.device_put(z, self.sharding) for z in self.zero_outs]
        outs = self.fn(*dev_in, *zo)
        self.jax.block_until_ready(outs)
        return outs

    def run_np(self, dev_in):
        outs = self.run(dev_in)
        return {
            name: np.asarray(outs[i]).reshape(NCORES, *self.out_avals[i].shape)
            for i, name in enumerate(self.out_names)
        }


def _get_runner(phase=None, reps=1):
    if phase is None:
        phase = int(os.environ.get("KERNEL_PHASE", "1"))
    key = f"runner_p{phase}_r{reps}"
    if key not in _CACHE:
        _CACHE[key] = _Runner(_get_nc(phase, reps))
    return _CACHE[key]


def make_in_maps(x, emb, idf, W1, b1, W2, b2, W3, b3, phase):
    xt = np.ascontiguousarray(np.asarray(x, dtype=np.int32).T)  # [B, S]
    emb = np.ascontiguousarray(np.asarray(emb, dtype=np.float32))
    idf = np.asarray(idf, dtype=np.float32)
    idf_pad = np.zeros(VPAD, dtype=np.float32)
    idf_pad[:V] = idf
    idf_pad[0] = 0.0  # pad token contributes nothing
    idf_t = np.ascontiguousarray(idf_pad.reshape(NHI, NLO).T)  # [256, 196]

    w1t = np.ascontiguousarray(np.asarray(W1, dtype=np.float32).T)
    w2t = np.ascontiguousarray(np.asarray(W2, dtype=np.float32).T)
    w3t = np.ascontiguousarray(np.asarray(W3, dtype=np.float32).T)
    b1 = np.ascontiguousarray(np.asarray(b1, dtype=np.float32))
    b2 = np.ascontiguousarray(np.asarray(b2, dtype=np.float32))
    b3 = np.ascontiguousarray(np.asarray(b3, dtype=np.float32))

    if phase == 2:
        emb_pad = np.zeros((NCORES * VSH, D), dtype=np.float32)
        emb_pad[:V] = emb
    in_maps = []
    for c in range(NCORES):
        m = {
            "xt": np.ascontiguousarray(xt[c * RPC : (c + 1) * RPC, :]),
            "idf_t": idf_t,
            "w1t": w1t, "b1": b1, "w2t": w2t, "b2": b2,
            "w3t": w3t, "b3": b3,
        }
        if phase == 1:
            m["emb"] = emb
        else:
            m["embs"] = emb_pad[c * VSH : (c + 1) * VSH]
        in_maps.append(m)
    return in_maps


def kernel(x, emb, idf, W1, b1, W2, b2, W3, b3):
    phase = int(os.environ.get("KERNEL_PHASE", "1"))
    in_maps = make_in_maps(x, emb, idf, W1, b1, W2, b2, W3, b3, phase)
    runner = _get_runner(phase)
    dev_in = runner.put_inputs(in_maps)
    _CACHE["last_dev_in"] = dev_in
    outs = runner.run_np(dev_in)
    outp = np.concatenate([outs["out"][c] for c in range(NCORES)], axis=0)
    return outp.astype(np.float32)



# revision 7
# speedup vs baseline: 4.5367x; 4.5367x over previous
"""Trainium2 Bass kernel for tf-idf weighted embedding pooling + MLP.

Math: per batch row b (64 rows), tf[b,s] = within-row count of token x[s,b];
scores = where(tok==0, 0, tf*idf[tok]); pooled[b] = sum_s scores * emb[tok];
out = softmax(relu(relu(pooled@W1.T+b1)@W2.T+b2)@W3.T+b3).

Key identity: pooled[b] = sum_t c_t^2 * idf_t * emb_t  (c_t = count of t in row b).
Histogram per row is computed on the PE as a one-hot digit matmul:
  tok = hi*256 + lo,  H[lo, hi] = OneHotLo^T @ OneHotHi  (accumulated over s)
so H[lo, hi] = count of token hi*256+lo.  Then a[lo, hi] = H^2 * idfT[lo, hi].

Phase 2 (default): vocab-sharded pooled matmul. Each core histograms its own
8 batch rows, AllToAll redistributes the a-vectors so core c holds a[all 64
rows, vocab shard c], each core contracts its 6400-row emb shard (zero-padded
past 50000) against a for all 64 rows, and a ReduceScatter hands core c the
final pooled rows 8c..8c+8 for its MLP slice. Per-core HBM read of emb drops
from 51.2MB to 6.4MB.

Phase 1 (fallback): every core streams the full 51.2MB emb table.
"""

import os
import sys

import numpy as np

sys.path.insert(0, "/opt/trn_rl_repo")

import concourse.bass as bass  # noqa: E402,F401
import concourse.mybir as mybir  # noqa: E402
import concourse.tile as tile  # noqa: E402
from concourse import bacc  # noqa: E402
from concourse.masks import make_identity  # noqa: E402

P = 128
S = 2048
B = 64
D = 256
V = 50000
NCORES = 8
RPC = B // NCORES  # rows per core
NHI = 196  # ceil(50000/256)
NLO = 256
STILES = S // P  # 16
VPAD = NHI * NLO  # 50176
NHL = 25  # hi rows per vocab shard (200 padded hi rows / 8)
VSH = NHL * NLO  # 6400 vocab rows per shard

F32 = mybir.dt.float32
BF16 = mybir.dt.bfloat16
F16 = mybir.dt.float16
I32 = mybir.dt.int32

# Phase 3: vocab split 128 x 392 (lo = t & 127 -> partition, hi = t >> 7 -> free)
NLO3 = 128
NHI3 = 392
NHL3 = NHI3 // NCORES  # 49 hi values per vocab shard
VSH3 = NHL3 * NLO3  # 6272 vocab rows per shard
VPAD3 = NLO3 * NHI3  # 50176

_CACHE = {}


def _mlp_tail(nc, tc, cpool, ps_mlp, pooled_sb, identity,
              w1t_sb, b1_sb, w2t_sb, b2a_sb, b2b_sb, w3a_sb, w3b_sb, b3_sb, out):
    """pooled_sb [RPC, 256] -> softmax out DMA."""
    pooledT = cpool.tile([P, 2, RPC], F32, tag="pooledT")
    for kc in range(2):
        ptp = ps_mlp.tile([P, RPC], F32, tag="ptp")
        nc.tensor.transpose(
            ptp[:, :], pooled_sb[:, kc * P : (kc + 1) * P], identity[:RPC, :RPC]
        )
        nc.vector.tensor_copy(pooledT[:, kc, :], ptp[:, :])

    h1_ps = ps_mlp.tile([100, RPC], F32, tag="h1")
    for kc in range(2):
        nc.tensor.matmul(
            h1_ps[:, :], lhsT=w1t_sb[:, kc, :], rhs=pooledT[:, kc, :],
            start=(kc == 0), stop=(kc == 1),
        )
    h1_sb = cpool.tile([100, RPC], F32, tag="h1_sb")
    nc.scalar.activation(
        h1_sb[:], h1_ps[:, :], mybir.ActivationFunctionType.Relu,
        bias=b1_sb[:, 0:1], scale=1.0,
    )

    h2a_ps = ps_mlp.tile([P, RPC], F32, tag="h2a")
    nc.tensor.matmul(h2a_ps[:, :], lhsT=w2t_sb[:, 0:128], rhs=h1_sb[:, :],
                     start=True, stop=True)
    h2b_ps = ps_mlp.tile([22, RPC], F32, tag="h2b")
    nc.tensor.matmul(h2b_ps[:, :], lhsT=w2t_sb[:, 128:150], rhs=h1_sb[:, :],
                     start=True, stop=True)
    h2a_sb = cpool.tile([P, RPC], F32, tag="h2a_sb")
    h2b_sb = cpool.tile([22, RPC], F32, tag="h2b_sb")
    nc.scalar.activation(h2a_sb[:], h2a_ps[:, :],
                         mybir.ActivationFunctionType.Relu,
                         bias=b2a_sb[:, 0:1], scale=1.0)
    nc.scalar.activation(h2b_sb[:], h2b_ps[:, :],
                         mybir.ActivationFunctionType.Relu,
                         bias=b2b_sb[:, 0:1], scale=1.0)

    lg_ps = ps_mlp.tile([2, RPC], F32, tag="lg")
    nc.tensor.matmul(lg_ps[:, :], lhsT=w3a_sb[:, :], rhs=h2a_sb[:, :],
                     start=True, stop=False)
    nc.tensor.matmul(lg_ps[:, :], lhsT=w3b_sb[:, :], rhs=h2b_sb[:, :],
                     start=False, stop=True)
    lg_sb = cpool.tile([2, RPC], F32, tag="lg_sb")
    nc.scalar.add(lg_sb[:], lg_ps[:, :], b3_sb[:, 0:1])

    lt_ps = ps_mlp.tile([RPC, 2], F32, tag="lt")
    nc.tensor.transpose(lt_ps[:, :], lg_sb[:, :], identity[:2, :2])
    e_sb = cpool.tile([RPC, 2], F32, tag="e_sb")
    nc.scalar.activation(e_sb[:], lt_ps[:, :], mybir.ActivationFunctionType.Exp)
    ssum = cpool.tile([RPC, 1], F32, tag="ssum")
    nc.vector.tensor_reduce(ssum[:], e_sb[:], axis=mybir.AxisListType.X,
                            op=mybir.AluOpType.add)
    rinv = cpool.tile([RPC, 1], F32, tag="rinv")
    nc.vector.reciprocal(rinv[:], ssum[:])
    res_sb = cpool.tile([RPC, 2], F32, tag="res_sb")
    nc.vector.tensor_scalar(out=res_sb[:], in0=e_sb[:], scalar1=rinv[:, 0:1],
                            scalar2=None, op0=mybir.AluOpType.mult)
    nc.sync.dma_start(out[:, :], res_sb[:])


def _build_nc(phase=1, reps=1):
    nc = bacc.Bacc(None, target_bir_lowering=False, debug=False)

    xt = nc.dram_tensor("xt", [RPC, S], I32, kind="ExternalInput")
    if phase == 1:
        emb = nc.dram_tensor("emb", [V, D], F32, kind="ExternalInput")
    else:
        embs = nc.dram_tensor("embs", [VSH, D], F32, kind="ExternalInput")
    idf_t = nc.dram_tensor("idf_t", [NLO, NHI], F32, kind="ExternalInput")
    w1t = nc.dram_tensor("w1t", [256, 100], F32, kind="ExternalInput")
    b1 = nc.dram_tensor("b1", [100], F32, kind="ExternalInput")
    w2t = nc.dram_tensor("w2t", [100, 150], F32, kind="ExternalInput")
    b2 = nc.dram_tensor("b2", [150], F32, kind="ExternalInput")
    w3t = nc.dram_tensor("w3t", [150, 2], F32, kind="ExternalInput")
    b3 = nc.dram_tensor("b3", [2], F32, kind="ExternalInput")
    out = nc.dram_tensor("out", [RPC, 2], F32, kind="ExternalOutput")

    with tile.TileContext(nc) as tc:
        with (
            tc.tile_pool(name="const", bufs=1) as cpool,
            tc.tile_pool(name="work", bufs=3) as wpool,
            tc.tile_pool(name="oh", bufs=6) as ohpool,
            tc.tile_pool(name="embp", bufs=17) as embpool,
            tc.tile_pool(name="arp", bufs=6) as arpool,
            tc.tile_pool(name="dram", bufs=2, space="DRAM") as dpool,
            tc.tile_pool(name="ps_acc", bufs=2, space="PSUM") as ps_acc,
        ):
            # ---------- constants ----------
            iota_i32 = cpool.tile([P, NLO], I32)
            nc.gpsimd.iota(iota_i32[:], pattern=[[1, NLO]], base=0,
                           channel_multiplier=0)
            iota_bf = cpool.tile([P, NLO], BF16)
            nc.vector.tensor_copy(iota_bf[:], iota_i32[:])

            identity = cpool.tile([P, P], F32)
            make_identity(nc, identity[:])

            idf_sb = cpool.tile([P, 2, NHI], F32)
            nc.sync.dma_start(idf_sb[:, 0, :], idf_t[0:128, :])
            nc.sync.dma_start(idf_sb[:, 1, :], idf_t[128:256, :])

            w1t_sb = cpool.tile([P, 2, 100], F32)
            nc.sync.dma_start(w1t_sb[:, :, :],
                              w1t[:, :].rearrange("(c p) m -> p c m", p=P))
            b1_sb = cpool.tile([100, 1], F32)
            nc.sync.dma_start(b1_sb[:, :], b1[:, None])
            w2t_sb = cpool.tile([100, 150], F32)
            nc.sync.dma_start(w2t_sb[:, :], w2t[:, :])
            b2a_sb = cpool.tile([128, 1], F32)
            b2b_sb = cpool.tile([22, 1], F32)
            nc.sync.dma_start(b2a_sb[:, :], b2[:128, None])
            nc.sync.dma_start(b2b_sb[:, :], b2[128:150, None])
            w3a_sb = cpool.tile([128, 2], F32)
            w3b_sb = cpool.tile([22, 2], F32)
            nc.sync.dma_start(w3a_sb[:, :], w3t[0:128, :])
            nc.sync.dma_start(w3b_sb[:, :], w3t[128:150, :])
            b3_sb = cpool.tile([2, 1], F32)
            nc.sync.dma_start(b3_sb[:, :], b3[:, None])

            for _rep in range(reps):
                # ---------- tokens: [128, RPC*16], s = p*16 + f per row ------
                tok_i32 = cpool.tile([P, RPC * STILES], I32, tag="tok", bufs=2)
                for r in range(RPC):
                    nc.sync.dma_start(
                        tok_i32[:, r * STILES : (r + 1) * STILES],
                        xt[r, :].rearrange("(p f) -> p f", p=P),
                    )
                lo_i32 = wpool.tile([P, RPC * STILES], I32, tag="lo_i32")
                hi_i32 = wpool.tile([P, RPC * STILES], I32, tag="hi_i32")
                nc.vector.tensor_scalar(
                    out=lo_i32[:], in0=tok_i32[:], scalar1=255, scalar2=None,
                    op0=mybir.AluOpType.bitwise_and)
                nc.vector.tensor_scalar(
                    out=hi_i32[:], in0=tok_i32[:], scalar1=8, scalar2=None,
                    op0=mybir.AluOpType.logical_shift_right)
                lo_f = cpool.tile([P, RPC * STILES], F32, tag="lo_f", bufs=2)
                hi_f = cpool.tile([P, RPC * STILES], F32, tag="hi_f", bufs=2)
                nc.vector.tensor_copy(lo_f[:], lo_i32[:])
                nc.vector.tensor_copy(hi_f[:], hi_i32[:])
                # negated hi for the ACT-engine one-hot (bias port)
                hi_neg = cpool.tile([P, RPC * STILES], F32, tag="hi_neg",
                                    bufs=2)
                nc.vector.tensor_scalar(
                    out=hi_neg[:], in0=hi_f[:], scalar1=-1.0, scalar2=None,
                    op0=mybir.AluOpType.mult)

                # ---------- per-row histograms -> a = H^2 * idfT ----------
                a_all = [cpool.tile([P, NHI, RPC], F32, name=f"a_all{mh}",
                                    tag=f"a_all{mh}", bufs=2)
                         for mh in range(2)]
                if phase == 2:
                    # AllToAll buffers: [hi(200=8sh*25), mh, p, r]
                    a2a_in = dpool.tile([8 * NHL, 2, P, RPC], F32,
                                        tag="a2a_in")
                    a2a_out = dpool.tile([8, NHL, 2, P, RPC], F32,
                                         tag="a2a_out")
                    # zero the hi 196..199 pad region (uninit DRAM garbage
                    # would ride the AllToAll and NaN-poison the matmul)
                    zpad = cpool.tile([P, (8 * NHL - NHI) * 2 * RPC], F32,
                                      tag="zpad", bufs=2)
                    nc.vector.memset(zpad[:], 0.0)
                    nc.sync.dma_start(
                        a2a_in[NHI:, :, :, :].rearrange(
                            "hi mh p r -> p hi mh r"),
                        zpad[:].rearrange("p (hi mh r) -> p hi mh r",
                                          hi=8 * NHL - NHI, mh=2),
                    )

                with tc.tile_pool(name="ps_ht", bufs=2, space="PSUM") as ps_ht:
                    for r in range(RPC):
                        ht_ps = [ps_ht.tile([P, NHI], F32, name=f"ht{mh}",
                                            tag=f"ht{mh}")
                                 for mh in range(2)]
                        for f in range(STILES):
                            col = r * STILES + f
                            lo_oh = ohpool.tile([P, NLO], BF16, tag="lo_oh")
                            hi_oh = ohpool.tile([P, NHI], BF16, tag="hi_oh")
                            if True:
                                nc.vector.tensor_scalar(
                                    out=lo_oh[:], in0=iota_bf[:],
                                    scalar1=lo_f[:, col : col + 1],
                                    scalar2=None,
                                    op0=mybir.AluOpType.is_equal)
                                d2 = ohpool.tile([P, NHI], F32,
                                                 tag="d2")
                                nc.scalar.activation(
                                    d2[:], iota_bf[:, :NHI],
                                    mybir.ActivationFunctionType.Square,
                                    bias=hi_neg[:, col : col + 1], scale=1.0)
                                nc.scalar.activation(
                                    hi_oh[:], d2[:],
                                    mybir.ActivationFunctionType.Relu,
                                    bias=1.0, scale=-1.0)
                            for mh in range(2):
                                nc.tensor.matmul(
                                    ht_ps[mh][:, :],
                                    lhsT=lo_oh[:, mh * P : (mh + 1) * P],
                                    rhs=hi_oh[:, :],
                                    start=(f == 0), stop=(f == STILES - 1))
                        for mh in range(2):
                            sq = wpool.tile([P, NHI], F32, tag="sq")
                            nc.scalar.square(sq[:], ht_ps[mh][:, :])
                            nc.vector.tensor_tensor(
                                out=a_all[mh][:, :, r], in0=sq[:],
                                in1=idf_sb[:, mh, :],
                                op=mybir.AluOpType.mult)

                if phase == 2:
                    # bulk-write a to the exchange buffer: one DMA per mh,
                    # innermost r contiguous (32B runs)
                    for mh in range(2):
                        nc.sync.dma_start(
                            a2a_in[:NHI, mh, :, :].rearrange(
                                "hi p r -> p hi r"),
                            a_all[mh][:, :, :],
                        )

                if phase == 1:
                    # ------- pooled over full-vocab chunks (M=RPC) -------
                    pooled_ps = ps_acc.tile([RPC, D], F32, tag="pooled")
                    chunks = []
                    v = 0
                    while v + 1024 <= 49152:
                        chunks.append((v, 1024))
                        v += 1024
                    chunks.append((49152, 768))
                    chunks.append((49920, 80))
                    n_mm = sum((n + 127) // 128 for _, n in chunks)
                    mm_i = 0
                    for v0, n in chunks:
                        embc = embpool.tile([P, 8, D], F32, tag="embc")
                        if n >= P:
                            nsub = n // P
                            nc.sync.dma_start(
                                embc[:, :nsub, :],
                                emb[v0 : v0 + nsub * P, :].rearrange(
                                    "(c p) d -> p c d", p=P))
                        else:
                            nsub = 1
                            nc.sync.dma_start(embc[:n, 0, :],
                                              emb[v0 : v0 + n, :])
                        for c in range(nsub):
                            vv = v0 + c * P
                            kk = min(P, n - c * P)
                            hi = vv >> 8
                            mh = (vv >> 7) & 1
                            nc.tensor.matmul(
                                pooled_ps[:, :],
                                lhsT=a_all[mh][:kk, hi, :],
                                rhs=embc[:kk, c, :],
                                start=(mm_i == 0), stop=(mm_i == n_mm - 1))
                            mm_i += 1
                    pooled_sb = cpool.tile([RPC, D], F32, tag="pooled_sb")
                    nc.vector.tensor_copy(pooled_sb[:], pooled_ps[:, :])
                else:
                    # ------- AllToAll, shard matmul (M=64), ReduceScatter ---
                    nc.gpsimd.collective_compute(
                        "AllToAll", mybir.AluOpType.bypass,
                        replica_groups=[list(range(NCORES))],
                        ins=[a2a_in[:, :, :, :]],
                        outs=[a2a_out[:, :, :, :, :]],
                    )
                    pooled_ps = ps_acc.tile([B, D], F32, tag="pooled")
                    # emb shard chunks of 1024 rows (6 full + 1x256)
                    ech = [(i * 1024, 1024) for i in range(6)] + [(6144, 256)]
                    mm_i = 0
                    for v0, n in ech:
                        embc = embpool.tile([P, 8, D], F32, tag="embc")
                        nsub = n // P
                        nc.sync.dma_start(
                            embc[:, :nsub, :],
                            embs[v0 : v0 + n, :].rearrange(
                                "(c p) d -> p c d", p=P))
                        for c in range(nsub):
                            k = v0 // P + c  # 0..49
                            hl, mh = k >> 1, k & 1
                            ar = arpool.tile([P, B], F32, tag="ar")
                            nc.sync.dma_start(
                                ar[:].rearrange("p (j r) -> p j r", j=8),
                                a2a_out[:, hl, mh, :, :].rearrange(
                                    "j p r -> p j r"))
                            nc.tensor.matmul(
                                pooled_ps[:, :], lhsT=ar[:],
                                rhs=embc[:, c, :],
                                start=(mm_i == 0), stop=(mm_i == 49))
                            mm_i += 1
                    pooled_full = cpool.tile([B, D], F32, tag="pooled_full",
                                             bufs=2)
                    nc.vector.tensor_copy(pooled_full[:], pooled_ps[:, :])
                    rs_in = dpool.tile([B, D], F32, tag="rs_in")
                    rs_out = dpool.tile([RPC, D], F32, tag="rs_out")
                    nc.sync.dma_start(rs_in[:, :], pooled_full[:])
                    nc.gpsimd.collective_compute(
                        "ReduceScatter", mybir.AluOpType.add,
                        replica_groups=[list(range(NCORES))],
                        ins=[rs_in[:, :]],
                        outs=[rs_out[:, :]],
                    )
                    pooled_sb = cpool.tile([RPC, D], F32, tag="pooled_sb")
                    nc.sync.dma_start(pooled_sb[:], rs_out[:, :])

                # ---------- MLP + softmax on own 8 rows ----------
                with tc.tile_pool(name="ps_mlp", bufs=1,
                                  space="PSUM") as ps_mlp:
                    _mlp_tail(nc, tc, cpool, ps_mlp, pooled_sb, identity,
                              w1t_sb, b1_sb, w2t_sb, b2a_sb, b2b_sb,
                              w3a_sb, w3b_sb, b3_sb, out)

    nc.compile()
    return nc


def _build_nc3(reps=1):
    """Phase 3: vocab-sharded pooled matmul with DMA-friendly layouts.

    Vocab id t = hi*128 + lo (lo = t & 127 -> SBUF partition, hi = t >> 7).
    Per core: one-hot histogram matmuls (fp16, one MM of N=392 per 128-token
    seq chunk) for its 8 batch rows; a = H^2 * idf cast to bf16; per-row DMA
    into the AllToAll exchange buffer (overlapped with later rows' hist).
    AllToAll hands core c a[all 64 rows, hi in 49c..49c+49); core c contracts
    its resident bf16 emb shard (6272 rows, loaded once at start) in 49
    matmuls of M=64/N=256; ReduceScatter sums partial pooled and scatters 8
    rows per core for the MLP tail.
    """
    nc = bacc.Bacc(None, target_bir_lowering=False, debug=False)

    xt = nc.dram_tensor("xt", [RPC, S], I32, kind="ExternalInput")
    embs = nc.dram_tensor("embs", [VSH3, D], BF16, kind="ExternalInput")
    idf_t = nc.dram_tensor("idf_t", [NLO3, NHI3], F32, kind="ExternalInput")
    w1t = nc.dram_tensor("w1t", [256, 100], F32, kind="ExternalInput")
    b1 = nc.dram_tensor("b1", [100], F32, kind="ExternalInput")
    w2t = nc.dram_tensor("w2t", [100, 150], F32, kind="ExternalInput")
    b2 = nc.dram_tensor("b2", [150], F32, kind="ExternalInput")
    w3t = nc.dram_tensor("w3t", [150, 2], F32, kind="ExternalInput")
    b3 = nc.dram_tensor("b3", [2], F32, kind="ExternalInput")
    out = nc.dram_tensor("out", [RPC, 2], F32, kind="ExternalOutput")

    with tile.TileContext(nc) as tc:
        with (
            tc.tile_pool(name="const", bufs=1) as cpool,
            tc.tile_pool(name="work", bufs=3) as wpool,
            tc.tile_pool(name="oh", bufs=6) as ohpool,
            tc.tile_pool(name="arow", bufs=3) as arpool,
            tc.tile_pool(name="dram", bufs=2, space="DRAM") as dpool,
            tc.tile_pool(name="ps_acc", bufs=2, space="PSUM") as ps_acc,
        ):
            # ---------- constants ----------
            iota_lo_i = cpool.tile([P, NLO3], I32)
            nc.gpsimd.iota(iota_lo_i[:], pattern=[[1, NLO3]], base=0,
                           channel_multiplier=0)
            iota_hi_i = cpool.tile([P, NHI3], I32)
            nc.gpsimd.iota(iota_hi_i[:], pattern=[[1, NHI3]], base=0,
                           channel_multiplier=0)
            iota_lo = cpool.tile([P, NLO3], F16)
            nc.vector.tensor_copy(iota_lo[:], iota_lo_i[:])
            iota_hi = cpool.tile([P, NHI3], F16)
            nc.vector.tensor_copy(iota_hi[:], iota_hi_i[:])

            identity = cpool.tile([P, P], F32)
            make_identity(nc, identity[:])

            idf_sb = cpool.tile([P, NHI3], F32)
            nc.sync.dma_start(idf_sb[:, :], idf_t[:, :])

            w1t_sb = cpool.tile([P, 2, 100], F32)
            nc.sync.dma_start(w1t_sb[:, :, :],
                              w1t[:, :].rearrange("(c p) m -> p c m", p=P))
            b1_sb = cpool.tile([100, 1], F32)
            nc.sync.dma_start(b1_sb[:, :], b1[:, None])
            w2t_sb = cpool.tile([100, 150], F32)
            nc.sync.dma_start(w2t_sb[:, :], w2t[:, :])
            b2a_sb = cpool.tile([128, 1], F32)
            b2b_sb = cpool.tile([22, 1], F32)
            nc.sync.dma_start(b2a_sb[:, :], b2[:128, None])
            nc.sync.dma_start(b2b_sb[:, :], b2[128:150, None])
            w3a_sb = cpool.tile([128, 2], F32)
            w3b_sb = cpool.tile([22, 2], F32)
            nc.sync.dma_start(w3a_sb[:, :], w3t[0:128, :])
            nc.sync.dma_start(w3b_sb[:, :], w3t[128:150, :])
            b3_sb = cpool.tile([2, 1], F32)
            nc.sync.dma_start(b3_sb[:, :], b3[:, None])

            # ---------- resident emb shard: [p, hl, d] bf16 (24.5KB/part) ----
            emb_sb = cpool.tile([P, NHL3, D], BF16)
            for g in range(7):
                h0 = g * 8
                nh = min(8, NHL3 - h0)
                nc.sync.dma_start(
                    emb_sb[:, h0 : h0 + nh, :],
                    embs[h0 * P : (h0 + nh) * P, :].rearrange(
                        "(c p) d -> p c d", p=P),
                )

            for _rep in range(reps):
                # tokens [128, (r f)]: seq pos s = p*16 + f of row r
                tok = cpool.tile([P, RPC, STILES], I32, tag="tok", bufs=2)
                nc.sync.dma_start(
                    tok[:, :, :],
                    xt[:, :].rearrange("r (p f) -> p r f", p=P),
                )
                lo_i = wpool.tile([P, RPC * STILES], I32, tag="lo_i")
                hi_i = wpool.tile([P, RPC * STILES], I32, tag="hi_i")
                tok_flat = tok[:, :, :].rearrange("p r f -> p (r f)")
                nc.vector.tensor_scalar(
                    out=lo_i[:], in0=tok_flat, scalar1=127, scalar2=None,
                    op0=mybir.AluOpType.bitwise_and)
                nc.vector.tensor_scalar(
                    out=hi_i[:], in0=tok_flat, scalar1=7, scalar2=None,
                    op0=mybir.AluOpType.logical_shift_right)
                lo_f = cpool.tile([P, RPC * STILES], F32, tag="lo_f", bufs=2)
                hi_f = cpool.tile([P, RPC * STILES], F32, tag="hi_f", bufs=2)
                nc.vector.tensor_copy(lo_f[:], lo_i[:])
                nc.vector.tensor_copy(hi_f[:], hi_i[:])

                # exchange buffers: a2a_in[c, p, r, hl] (shard c block)
                a2a_in = dpool.tile([NCORES, P, RPC, NHL3], BF16,
                                    tag="a2a_in")
                a2a_out = dpool.tile([NCORES, P, RPC, NHL3], BF16,
                                     tag="a2a_out")

                # ---------- per-row histogram -> a -> exchange ----------
                with tc.tile_pool(name="ps_ht", bufs=2, space="PSUM") as ps_ht:
                    for r in range(RPC):
                        ht_ps = ps_ht.tile([P, NHI3], F32, tag="ht")
                        for f in range(STILES):
                            col = r * STILES + f
                            lo_oh = ohpool.tile([P, NLO3], F16, tag="lo_oh")
                            hi_oh = ohpool.tile([P, NHI3], F16, tag="hi_oh")
                            nc.vector.tensor_scalar(
                                out=lo_oh[:], in0=iota_lo[:],
                                scalar1=lo_f[:, col : col + 1], scalar2=None,
                                op0=mybir.AluOpType.is_equal)
                            nc.vector.tensor_scalar(
                                out=hi_oh[:], in0=iota_hi[:],
                                scalar1=hi_f[:, col : col + 1], scalar2=None,
                                op0=mybir.AluOpType.is_equal)
                            nc.tensor.matmul(
                                ht_ps[:, :], lhsT=lo_oh[:], rhs=hi_oh[:],
                                start=(f == 0), stop=(f == STILES - 1))
                        sq = wpool.tile([P, NHI3], F32, tag="sq")
                        nc.scalar.square(sq[:], ht_ps[:, :])
                        a_row = arpool.tile([P, NCORES, NHL3], BF16,
                                            tag="a_row")
                        nc.vector.tensor_tensor(
                            out=a_row[:, :, :].rearrange("p c hl -> p (c hl)"),
                            in0=sq[:], in1=idf_sb[:],
                            op=mybir.AluOpType.mult)
                        nc.sync.dma_start(
                            a2a_in[:, :, r, :].rearrange("c p hl -> p c hl"),
                            a_row[:, :, :])

                # ---------- AllToAll + shard matmul + ReduceScatter ----------
                nc.gpsimd.collective_compute(
                    "AllToAll", mybir.AluOpType.bypass,
                    replica_groups=[list(range(NCORES))],
                    ins=[a2a_in[:, :, :, :]],
                    outs=[a2a_out[:, :, :, :]],
                )
                aT = cpool.tile([P, NCORES, RPC, NHL3], BF16, tag="aT",
                                bufs=2)
                nc.sync.dma_start(
                    aT[:, :, :, :],
                    a2a_out[:, :, :, :].rearrange("j p r hl -> p j r hl"))
                pooled_ps = ps_acc.tile([B, D], F32, tag="pooled")
                for hl in range(NHL3):
                    nc.tensor.matmul(
                        pooled_ps[:, :], lhsT=aT[:, :, :, hl],
                        rhs=emb_sb[:, hl, :],
                        start=(hl == 0), stop=(hl == NHL3 - 1))
                pooled_full = cpool.tile([B, D], F32, tag="pooled_full",
                                         bufs=2)
                nc.vector.tensor_copy(pooled_full[:], pooled_ps[:, :])
                rs_in = dpool.tile([B, D], F32, tag="rs_in")
                rs_out = dpool.tile([RPC, D], F32, tag="rs_out")
                nc.sync.dma_start(rs_in[:, :], pooled_full[:])
                nc.gpsimd.collective_compute(
                    "ReduceScatter", mybir.AluOpType.add,
                    replica_groups=[list(range(NCORES))],
                    ins=[rs_in[:, :]],
                    outs=[rs_out[:, :]],
                )
                pooled_sb = cpool.tile([RPC, D], F32, tag="pooled_sb",
                                       bufs=2)
                nc.sync.dma_start(pooled_sb[:], rs_out[:, :])

                with tc.tile_pool(name="ps_mlp", bufs=1,
                                  space="PSUM") as ps_mlp:
                    _mlp_tail(nc, tc, cpool, ps_mlp, pooled_sb, identity,
                              w1t_sb, b1_sb, w2t_sb, b2a_sb, b2b_sb,
                              w3a_sb, w3b_sb, b3_sb, out)

    nc.compile()
    return nc


def _get_nc(phase=1, reps=1):
    key = f"nc_p{phase}_r{reps}"
    if key not in _CACHE:
        if phase == 3:
            _CACHE[key] = _build_nc3(reps)
        else:
            _CACHE[key] = _build_nc(phase, reps)
    return _CACHE[key]


class _Runner:
    """Cached jitted shard_map over the NEFF custom call (mirrors
    bass2jax.run_bass_via_pjrt, but reusable with device-resident inputs)."""

    def __init__(self, nc):
        import jax
        from jax.experimental.shard_map import shard_map
        from jax.sharding import Mesh, NamedSharding, PartitionSpec

        from concourse import bass2jax

        bass2jax.install_neuronx_cc_hook()
        assert nc.dbg_addr is None
        partition_name = (
            nc.partition_id_tensor.name if nc.partition_id_tensor else None
        )
        self._nc = nc
        self._partition_name = partition_name

        self.jax = jax
        in_names, out_names, out_avals, zero_outs = [], [], [], []
        for alloc in nc.m.functions[0].allocations:
            if not isinstance(alloc, mybir.MemoryLocationSet):
                continue
            name = alloc.memorylocations[0].name
            if alloc.kind == "ExternalInput":
                if name == partition_name:
                    continue
                in_names.append(name)
            elif alloc.kind == "ExternalOutput":
                out_names.append(name)
                shape = tuple(alloc.tensor_shape)
                dtype = mybir.dt.np(alloc.dtype)
                out_avals.append(jax.core.ShapedArray(shape, dtype))
                zero_outs.append(np.zeros((NCORES * shape[0], *shape[1:]), dtype))
        self.in_names = list(in_names)
        self.out_names = out_names
        self.out_avals = out_avals
        self.zero_outs = zero_outs
        n_params = len(in_names)
        n_outs = len(out_names)
        bind_names = tuple(
            in_names + out_names + ([partition_name] if partition_name else [])
        )
        donate = tuple(range(n_params, n_params + n_outs))

        def _body(*args):
            operands = list(args)
            if partition_name is not None:
                operands.append(bass2jax.partition_id_tensor())
            outs = bass2jax._bass_exec_p.bind(
                *operands,
                out_avals=tuple(out_avals),
                in_names=bind_names,
                out_names=tuple(out_names),
                lowering_input_output_aliases=(),
                sim_require_finite=True,
                sim_require_nnan=True,
                nc=nc,
            )
            return tuple(outs)

        devices = jax.devices()[:NCORES]
        self.mesh = Mesh(np.asarray(devices), ("core",))
        self.sharding = NamedSharding(self.mesh, PartitionSpec("core"))
        in_specs = (PartitionSpec("core"),) * (n_params + n_outs)
        out_specs = (PartitionSpec("core"),) * n_outs
        self.fn = jax.jit(
            shard_map(
                _body,
                mesh=self.mesh,
                in_specs=in_specs,
                out_specs=out_specs,
                check_rep=False,
            ),
            donate_argnums=donate,
            keep_unused=True,
        )

    def put_inputs(self, in_maps):
        concat = [
            np.concatenate([np.asarray(m[name]) for m in in_maps], axis=0)
            for name in self.in_names
        ]
        return [self.jax.device_put(a, self.sharding) for a in concat]

    def run(self, dev_in):
        zo = [self.jax.device_put(z, self.sharding) for z in self.zero_outs]
        outs = self.fn(*dev_in, *zo)
        self.jax.block_until_ready(outs)
        return outs

    def run_np(self, dev_in):
        outs = self.run(dev_in)
        return {
            name: np.asarray(outs[i]).reshape(NCORES, *self.out_avals[i].shape)
            for i, name in enumerate(self.out_names)
        }


def _get_runner(phase=None, reps=1):
    if phase is None:
        phase = int(os.environ.get("KERNEL_PHASE", "3"))
    key = f"runner_p{phase}_r{reps}"
    if key not in _CACHE:
        _CACHE[key] = _Runner(_get_nc(phase, reps))
    return _CACHE[key]


def make_in_maps(x, emb, idf, W1, b1, W2, b2, W3, b3, phase):
    xt = np.ascontiguousarray(np.asarray(x, dtype=np.int32).T)  # [B, S]
    emb = np.ascontiguousarray(np.asarray(emb, dtype=np.float32))
    idf = np.asarray(idf, dtype=np.float32)
    if phase == 3:
        import ml_dtypes

        idf_pad = np.zeros(VPAD3, dtype=np.float32)
        idf_pad[:V] = idf
        idf_pad[0] = 0.0  # pad token contributes nothing
        # idf_t[lo, hi] = idf[hi*128 + lo]
        idf_t3 = np.ascontiguousarray(idf_pad.reshape(NHI3, NLO3).T)
        emb_pad = np.zeros((VPAD3, D), dtype=np.float32)
        emb_pad[:V] = emb
        emb_bf = emb_pad.astype(ml_dtypes.bfloat16)

        w1t = np.ascontiguousarray(np.asarray(W1, dtype=np.float32).T)
        w2t = np.ascontiguousarray(np.asarray(W2, dtype=np.float32).T)
        w3t = np.ascontiguousarray(np.asarray(W3, dtype=np.float32).T)
        b1 = np.ascontiguousarray(np.asarray(b1, dtype=np.float32))
        b2 = np.ascontiguousarray(np.asarray(b2, dtype=np.float32))
        b3 = np.ascontiguousarray(np.asarray(b3, dtype=np.float32))
        in_maps = []
        for c in range(NCORES):
            in_maps.append({
                "xt": np.ascontiguousarray(xt[c * RPC : (c + 1) * RPC, :]),
                "embs": np.ascontiguousarray(
                    emb_bf[c * VSH3 : (c + 1) * VSH3]),
                "idf_t": idf_t3,
                "w1t": w1t, "b1": b1, "w2t": w2t, "b2": b2,
                "w3t": w3t, "b3": b3,
            })
        return in_maps

    idf_pad = np.zeros(VPAD, dtype=np.float32)
    idf_pad[:V] = idf
    idf_pad[0] = 0.0  # pad token contributes nothing
    idf_t = np.ascontiguousarray(idf_pad.reshape(NHI, NLO).T)  # [256, 196]

    w1t = np.ascontiguousarray(np.asarray(W1, dtype=np.float32).T)
    w2t = np.ascontiguousarray(np.asarray(W2, dtype=np.float32).T)
    w3t = np.ascontiguousarray(np.asarray(W3, dtype=np.float32).T)
    b1 = np.ascontiguousarray(np.asarray(b1, dtype=np.float32))
    b2 = np.ascontiguousarray(np.asarray(b2, dtype=np.float32))
    b3 = np.ascontiguousarray(np.asarray(b3, dtype=np.float32))

    if phase == 2:
        emb_pad = np.zeros((NCORES * VSH, D), dtype=np.float32)
        emb_pad[:V] = emb
    in_maps = []
    for c in range(NCORES):
        m = {
            "xt": np.ascontiguousarray(xt[c * RPC : (c + 1) * RPC, :]),
            "idf_t": idf_t,
            "w1t": w1t, "b1": b1, "w2t": w2t, "b2": b2,
            "w3t": w3t, "b3": b3,
        }
        if phase == 1:
            m["emb"] = emb
        else:
            m["embs"] = emb_pad[c * VSH : (c + 1) * VSH]
        in_maps.append(m)
    return in_maps


def kernel(x, emb, idf, W1, b1, W2, b2, W3, b3):
    phase = int(os.environ.get("KERNEL_PHASE", "3"))
    in_maps = make_in_maps(x, emb, idf, W1, b1, W2, b2, W3, b3, phase)
    runner = _get_runner(phase)
    dev_in = runner.put_inputs(in_maps)
    _CACHE["last_dev_in"] = dev_in
    outs = runner.run_np(dev_in)
    outp = np.concatenate([outs["out"][c] for c in range(NCORES)], axis=0)
    return outp.astype(np.float32)

